# revision 26
# baseline (speedup 1.0000x reference)
"""Trainium2 Bass kernel for ContinuousAttentiveStatisticsPooling.

Shape config (hardcoded): B=8, C=256, L=8192, A=128, 8 NeuronCores.

Length-balanced sharding: lengths are ~U(0.5,1), so instead of one
example per core (every core paying full L), the valid column ranges of
ALL examples are cut into W=896-wide slots and dealt evenly across the
8 cores (56 slots -> 7 per core for the reference lengths). Partial
softmax statistics (Z, S1, S2) are linear in l, so the host merges the
per-slot partials per example and runs the (tiny) finalize math in
numpy - the device streams slots and dumps raw accumulators only.

Math restructure (per example, x is [C, L]):
  - Host zeroes x beyond the valid length -> all L-reductions over
    processed columns equal masked reductions plus an exactly-known
    pinv contribution from processed-but-invalid (zero) columns.
  - Host folds input-moment/weight-only terms:
      gmean = sum(x)/total ; gstd = sqrt(clip(sum(x^2)/total - gmean^2))
      ch   = Wt2 @ gmean + Wt3 @ gstd + b_tdnn          (relu bias)
      cv   = W2 @ gmean + W3 @ gstd + b_val             (values const)
      pinv = exp(Wc' @ relu(ch))                        (invalid-col p)
  - Device streams its slots once (slot k of example b):
      vraw   = W1 @ x                   (values, raw: cv added on host)
      h      = relu(Wt1 @ x + ch_b)
      p      = exp(Wc' @ h)             (score bias b' dropped: a
                                         per-channel constant cancels in
                                         the softmax over L)
      Z[k] += p ; S1[k] += p*vraw ; S2[k] += (p*vraw)*vraw
  - Host: Z_b = sum_slots Z - n_invalid_b * pinv_b ;
      amean = S1/Z + cv ; avar = S2/Z - (S1/Z)^2 ; astd = sqrt(avar)

Schedule notes (from HW traces):
  - x and the x-side weights are fp8 e4m3; the v and ph matmuls run in
    DoubleRow mode (one matmul covers the whole K=256 contraction),
    which halved PE busy time and x DMA bytes. Scores matmul stays
    bf16. rms rel err ~2.6e-3 (gate 2e-2).
  - SBUF DMA cost here is per-row (descriptor) bound, so chs + all
    weights + x slot 0 ride ONE combined [128, 3.1KB] byte tensor
    (bitcast/rearranged into typed views), partition-halved across the
    sync and gpsimd rings; later slots alternate whole on those rings.
  - Slot 0's relu/exp/STT chain is split into bank-aligned column
    pieces (512/384) so DVE streaming starts as soon as the first
    half lands.
  - PSUM = v(2 tiles x 2 banks) + ph/s(2 x 2) = 8 banks; [128,1024]
    fp32 tiles used 896 wide so matmul halves (512, 384) stay
    bank-aligned.
  - The DVE stream (4 STTs/slot at 1x - the v operand is fp32 PSUM,
    which caps any DVE op at 1 elem/cycle) is the pacing engine
    (~29us); ACT ~24us, PE ~20us, both hidden under it.
  - Fixed costs: ~1us preamble + ~3us DMA queue/event latency + ~10us
    walrus event-teardown storm after the final barrier (constant,
    independent of program size).
  - No PE warmup matmuls: measured twice, they run slow (~700ns),
    delay the first chain, and couple into the v PSUM pool (+8us).
  - exp only (no Sqrt): Exp/Relu share one ACT table set; finalize
    math (sqrt etc.) lives on the host.
"""

import sys

if "/opt/trn_rl_repo" not in sys.path:
    sys.path.insert(0, "/opt/trn_rl_repo")

import numpy as np
import ml_dtypes

import concourse.bass as bass
import concourse.mybir as mybir
import concourse.tile as tile
from concourse.bass_utils import run_bass_kernel_spmd

B, C, L, A = 8, 256, 8192, 128
CB = C // 128          # 2 c-blocks
W = 896                # slot width (columns per streaming superblock)
H0, H1 = 512, 384      # matmul halves, bank-aligned in PSUM
NCORES = 8
EPS = 1e-12
F32 = mybir.dt.float32
BF16 = mybir.dt.bfloat16
FP8 = mybir.dt.float8e4
NP_FP8 = ml_dtypes.float8_e4m3
DR = mybir.MatmulPerfMode.DoubleRow
ALU = mybir.AluOpType
ACT = mybir.ActivationFunctionType

_mw_ctr = [0]


def _split_multiwaits(nc):
    """This walrus build supports only ONE sync-wait per instruction.
    Split multi-wait instructions into single-wait NoOps on the same engine
    (same-engine program order preserves semantics exactly)."""
    for f in nc.m.functions:
        for blk in f.blocks:
            insts = blk.instructions
            out = []
            changed = False
            for inst in insts:
                si = inst.sync_info
                if si is not None and len(si.on_wait) > 1:
                    changed = True
                    waits = list(si.on_wait)
                    for w in waits[:-1]:
                        _mw_ctr[0] += 1
                        nop = mybir.InstNoOp(
                            name=f"mwsplit-{_mw_ctr[0]}", ins=[], outs=[]
                        )
                        nop.engine = inst.engine
                        nop.sync_info = mybir.SyncInfo(on_wait=[w], on_update=[])
                        out.append(nop)
                    inst.sync_info = mybir.SyncInfo(
                        on_wait=[waits[-1]], on_update=list(si.on_update)
                    )
                out.append(inst)
            if changed:
                insts[:] = out


def _build_nc(n_sb):
    nc = bass.Bass()
    # x slots 1..n-1: [128, n_sb-1, CB, W] fp8, one slot = a contiguous
    # 2*W = 1.75KB per-partition DMA chunk; the x-side matmuls run in
    # fp8 DoubleRow mode (whole K=256 contraction in one matmul).
    # DMA cost here is per-ROW (descriptor) bound, so everything slot 0
    # needs - chs, wtt, wv1t, wct, x0 - rides ONE combined byte tensor
    # (one 128-row transfer, split across both rings by partition half).
    CMB = 4 * n_sb + 256 + 512 + 512 + CB * W
    cmb_d = nc.dram_tensor("cmb", [128, CMB], FP8, kind="ExternalInput")
    x_d = nc.dram_tensor("xp", [128, n_sb - 1, CB, W], FP8, kind="ExternalInput")
    # raw accumulators out: [(stat,cb) = 6, piece] (slot 0 = 2 pieces)
    n_pc = n_sb + 1
    out_d = nc.dram_tensor("out", [128, 6 * n_pc], F32, kind="ExternalOutput")

    with tile.TileContext(nc) as tc:
        with (
            tc.tile_pool(name="consts", bufs=1) as cp,
            tc.tile_pool(name="xs", bufs=1) as xp,
            tc.tile_pool(name="hw", bufs=3) as hp,
            tc.tile_pool(name="pw", bufs=4) as pp,
            tc.tile_pool(name="qw", bufs=4) as qp,
            tc.tile_pool(name="q2w", bufs=2) as q2p,
            tc.tile_pool(name="vcw", bufs=2) as vcp,
        ):
            # ---- DMAs across THREE in-order rings (sync / gpsimd /
            # scalar): slot 0 rides the otherwise-idle scalar ring so
            # streaming starts early; weights lead their ring. ----
            zz = cp.tile([128, 1], F32, tag="zz", name="zz")
            nc.vector.memset(zz, 0)
            zzo = cp.tile([128, 1], F32, tag="zzo", name="zzo")
            # dummy activation: forces the ACT table load at t~0
            nc.scalar.activation(out=zzo, in_=zz, func=ACT.Relu)


            # combined first transfer: [chs | wtt | wv1t | wct | x0],
            # partition-halved across the two fast rings; later slots
            # alternate rings behind it.
            cmb = cp.tile([128, CMB], FP8, tag="cmb", name="cmb")
            nc.sync.dma_start(out=cmb[0:64, :], in_=cmb_d[0:64, :])
            nc.gpsimd.dma_start(out=cmb[64:128, :], in_=cmb_d[64:128, :])
            o0 = 4 * n_sb
            chs = cmb[:, 0 : o0].bitcast(F32)
            wtt = cmb[:, o0 : o0 + 256].rearrange("p (k m) -> p k m", k=2)
            wv1t = cmb[:, o0 + 256 : o0 + 768].rearrange(
                "p (k c m) -> p k c m", k=2, c=CB)
            wct = cmb[:, o0 + 768 : o0 + 1280].bitcast(BF16).rearrange(
                "p (c m) -> p c m", c=CB)
            x0v = cmb[:, o0 + 1280 : o0 + 1280 + CB * W].rearrange(
                "p (c w) -> p c w", c=CB)

            xs = [x0v] + [xp.tile([128, CB, W], FP8, tag=f"x_{k}", name=f"x_{k}")
                          for k in range(1, n_sb)]
            for k in range(1, n_sb):
                eng = nc.gpsimd if k % 2 == 1 else nc.sync
                eng.dma_start(out=xs[k], in_=x_d[:, k - 1, :, :])

            # streaming accumulators (2D tiles: accum_out must be 2D)
            # layout matches out_d: [(stat,cb), piece]
            stat = cp.tile([128, 6 * n_pc], F32, tag="stat", name="stat")

            def acc(stat_i, cb, pi):
                col = (stat_i * CB + cb) * n_pc + pi
                return stat[:, col : col + 1]

            with (
                tc.tile_pool(name="psv", bufs=2, space="PSUM") as ps_v,
                tc.tile_pool(name="pss", bufs=2, space="PSUM") as ps_s,
            ):
                halves = [slice(0, H0), slice(H0, W)]

                def emit_ph(k):
                    # [128,1024] alloc keeps matmul outputs bank-aligned;
                    # DoubleRow: one matmul covers both 128-kblocks
                    ph = ps_s.tile([128, 1024], F32, tag="s", name="ph")
                    for hsl in halves:
                        nc.tensor.matmul(ph[:, hsl], lhsT=wtt[:, :, :],
                                         rhs=xs[k][:, :, hsl],
                                         start=True, stop=True, perf_mode=DR)
                    return ph

                def emit_v(k, cb):
                    vps = ps_v.tile([128, 1024], F32, tag="v", name="v")
                    for hsl in halves:
                        nc.tensor.matmul(vps[:, hsl], lhsT=wv1t[:, :, cb, :],
                                         rhs=xs[k][:, :, hsl],
                                         start=True, stop=True, perf_mode=DR)
                    return vps

                ph_next = emit_ph(0)
                v_next = {cb: emit_v(0, cb) for cb in range(CB)}

                # slot 0 runs in two bank-aligned column pieces so the
                # relu/exp/STT chain starts as soon as the first half of
                # x slot 0 lands (pipeline fill); later slots run whole.
                # Each piece gets its own accumulator column (host sums
                # piece partials per slot anyway).
                pieces = [(0, slice(0, H0)), (0, slice(H0, W))] + [
                    (k, slice(0, W)) for k in range(1, n_sb)
                ]

                ph_cur = None
                v_cur = None
                for pi, (k, csl) in enumerate(pieces):
                    w = csl.stop - csl.start
                    if pi == 0 or k != pieces[pi - 1][0]:
                        ph_cur = ph_next
                        v_cur = v_next
                    h = hp.tile([128, w], BF16, tag="h", name=f"h{pi}")
                    nc.scalar.activation(out=h, in_=ph_cur[:, csl],
                                         func=ACT.Relu, bias=chs[:, k : k + 1])
                    # bank-aligned s-matmul spans within this piece
                    spans = [(a - csl.start, b - csl.start)
                             for (a, b) in ((0, H0), (H0, W))
                             if a >= csl.start and b <= csl.stop]
                    for cb in range(CB):
                        # rebalance: for a few cb-slots ACT converts v to
                        # bf16 SBUF so both STTs run in the DVE 2x mode
                        # (all-2-byte operands); DVE is the pacing engine
                        # and ACT has slack.
                        use_conv = (cb == 0 and 1 <= k <= 4)
                        if use_conv:
                            vc = vcp.tile([128, w], BF16, tag="vc",
                                          name=f"vc{pi}")
                            nc.scalar.copy(vc, v_cur[cb][:, csl])
                            v_in = vc
                        else:
                            v_in = v_cur[cb][:, csl]
                        sps = ps_s.tile([128, 1024], F32, tag="s",
                                        name=f"s{pi}_{cb}")
                        for a, b in spans:
                            nc.tensor.matmul(sps[:, a:b], lhsT=wct[:, cb, :],
                                             rhs=h[:, a:b], start=True,
                                             stop=True)
                        if cb == 0 and pi + 1 < len(pieces) and \
                                pieces[pi + 1][0] != k:
                            ph_next = emit_ph(pieces[pi + 1][0])
                        p = pp.tile([128, w], BF16, tag="p", name=f"p{pi}")
                        nc.scalar.activation(
                            out=p, in_=sps[:, 0:w], func=ACT.Exp,
                            accum_out=acc(0, cb, pi),
                        )
                        q = qp.tile([128, w], BF16, tag="q", name=f"q{pi}")
                        nc.vector.scalar_tensor_tensor(
                            out=q, in0=p, scalar=0.0, in1=v_in,
                            op0=ALU.bypass, op1=ALU.mult,
                            accum_out=acc(1, cb, pi),
                        )
                        q2 = q2p.tile([128, w], BF16, tag="q2", name=f"q2{pi}")
                        nc.vector.scalar_tensor_tensor(
                            out=q2, in0=q, scalar=0.0, in1=v_in,
                            op0=ALU.bypass, op1=ALU.mult,
                            accum_out=acc(2, cb, pi),
                        )
                    if pi + 1 < len(pieces) and pieces[pi + 1][0] != k:
                        v_next = {cb: emit_v(pieces[pi + 1][0], cb)
                                  for cb in range(CB)}

            nc.sync.dma_start(out=out_d[:, :], in_=stat)

    _split_multiwaits(nc)
    return nc


_NC_CACHE = {}


def _get_nc(n_sb):
    if n_sb not in _NC_CACHE:
        _NC_CACHE[n_sb] = _build_nc(n_sb)
    return _NC_CACHE[n_sb]


def _prep_inputs(x, lengths, w_val, b_val, w_tdnn, b_tdnn, bn_gamma, bn_beta,
                 w_conv, b_conv):
    x = np.asarray(x, dtype=np.float32)
    lengths = np.asarray(lengths, dtype=np.float32)
    w_val = np.asarray(w_val, dtype=np.float32)
    b_val = np.asarray(b_val, dtype=np.float32)
    w_tdnn = np.asarray(w_tdnn, dtype=np.float32)
    b_tdnn = np.asarray(b_tdnn, dtype=np.float32)
    bn_gamma = np.asarray(bn_gamma, dtype=np.float32)
    bn_beta = np.asarray(bn_beta, dtype=np.float32)
    w_conv = np.asarray(w_conv, dtype=np.float32)
    b_conv = np.asarray(b_conv, dtype=np.float32)

    mask = (np.arange(L, dtype=np.float32)[None, :] < (lengths * L)[:, None])
    total = mask.sum(axis=1).astype(np.int64)               # [B]
    xmf = x * mask[:, None, :].astype(np.float32)
    xm = xmf.astype(NP_FP8)                                 # device copy

    # masked global moments (exact x; only the host uses these)
    totf = total.astype(np.float32)
    gmean = xmf.sum(axis=2) / totf[:, None]                                  # [B, C]
    gsq = (xmf * xmf).sum(axis=2) / totf[:, None]
    gstd = np.sqrt(np.clip(gsq - gmean * gmean, EPS, None))                  # [B, C]

    def pack_lhsT(w, kblocks, cblocks, dt=None):
        # w: [K, M] (contraction-major) -> [128, kblocks, cblocks, 128]
        Ktot, Mtot = w.shape
        assert Ktot == kblocks * 128 and Mtot == cblocks * 128
        r = np.ascontiguousarray(
            w.reshape(kblocks, 128, cblocks, 128).transpose(1, 0, 2, 3)
        )
        return r.astype(dt) if dt is not None else r

    W1T = w_val[:, :C].T                                   # [f, c]
    wv1t = pack_lhsT(W1T, 2, CB, NP_FP8)
    WtT = w_tdnn[:, :C].T                                  # [f, a]
    wtt = pack_lhsT(WtT, 2, 1, NP_FP8).reshape(128, 2, 128)
    WcT = (w_conv * bn_gamma[None, :]).T                   # [a, c] (BN gamma folded)
    wct = pack_lhsT(WcT, 1, CB, ml_dtypes.bfloat16).reshape(128, CB, 128)
    # score bias b' = b_conv + w_conv @ bn_beta is constant per channel
    # -> cancels in the softmax; not needed anywhere.

    # per-example folded consts
    chs_b = np.empty((B, A), np.float32)
    cv_b = np.empty((B, C), np.float32)
    pinv_b = np.empty((B, C), np.float32)
    for b in range(B):
        gcat = np.concatenate([gmean[b], gstd[b]])                           # [2C]
        ch = w_tdnn[:, C:] @ gcat + b_tdnn                                   # [A]
        cv_b[b] = w_val[:, C:] @ gcat + b_val                                # [C]
        hinv = np.maximum(ch, 0.0).astype(ml_dtypes.bfloat16).astype(np.float32)
        pinv_b[b] = WcT.astype(ml_dtypes.bfloat16).astype(np.float32).T @ hinv
        chs_b[b] = ch
    pinv_b = np.exp(pinv_b)

    # ---- slot assignment: cut valid ranges into W-wide slots, deal
    # round-robin across cores ----
    slots = []                                              # (b, l0, width)
    for b in range(B):
        l0 = 0
        while l0 < total[b]:
            slots.append((b, l0, int(min(W, total[b] - l0))))
            l0 += W
    n_sb = (len(slots) + NCORES - 1) // NCORES

    wbytes = np.concatenate([
        np.ascontiguousarray(wtt).reshape(128, 256).view(np.uint8),
        np.ascontiguousarray(wv1t).reshape(128, 512).view(np.uint8),
        np.ascontiguousarray(wct).reshape(128, 256).view(np.uint8),
    ], axis=1)                                              # [128, 1280]
    in_maps = []
    slot_map = []                                           # per core: [(b, width)]
    for core in range(NCORES):
        mine = slots[core::NCORES]
        xp = np.zeros((128, n_sb, CB, W), dtype=NP_FP8)
        chs = np.zeros((128, n_sb), dtype=np.float32)
        smap = []
        for k, (b, l0, w) in enumerate(mine):
            sl = xm[b, :, l0 : l0 + w].reshape(CB, 128, w)
            xp[:, k, :, :w] = sl.transpose(1, 0, 2)
            chs[:, k] = chs_b[b]
            smap.append((b, w))
        cmb = np.concatenate([
            chs.view(np.uint8), wbytes,
            np.ascontiguousarray(xp[:, 0]).reshape(128, CB * W).view(np.uint8),
        ], axis=1).view(NP_FP8)                             # [128, CMB]
        m = {"cmb": np.ascontiguousarray(cmb),
             "xp": np.ascontiguousarray(xp[:, 1:])}
        in_maps.append(m)
        slot_map.append(smap)
    return in_maps, n_sb, slot_map, total, cv_b, pinv_b


def kernel(**inputs) -> np.ndarray:
    in_maps, n_sb, slot_map, total, cv_b, pinv_b = _prep_inputs(**inputs)
    nc = _get_nc(n_sb)
    res = run_bass_kernel_spmd(nc, in_maps, core_ids=list(range(NCORES)))
    # merge per-slot partials per example (host-side finalize)
    Z = np.zeros((B, C), np.float64)
    S1 = np.zeros((B, C), np.float64)
    S2 = np.zeros((B, C), np.float64)
    nproc = np.zeros(B, np.int64)
    n_pc = n_sb + 1
    for core in range(NCORES):
        o = res.results[core]["out"].astype(np.float64)     # [128, 6*n_pc]
        o = o.reshape(128, 3, CB, n_pc)
        for k, (b, w) in enumerate(slot_map[core]):
            # slot 0 is split into piece columns 0 and 1
            cols = [0, 1] if k == 0 else [k + 1]
            for pi in cols:
                # stat columns are [c-block major] -> channel = cb*128 + p
                Z[b] += o[:, 0, :, pi].T.reshape(C)
                S1[b] += o[:, 1, :, pi].T.reshape(C)
                S2[b] += o[:, 2, :, pi].T.reshape(C)
            nproc[b] += W
    n_inv = (nproc - total).astype(np.float64)              # zero-padded cols
    Zv = Z - n_inv[:, None] * pinv_b.astype(np.float64)
    m1 = S1 / Zv
    amean = m1 + cv_b
    avar = np.clip(S2 / Zv - m1 * m1, EPS, None)
    astd = np.sqrt(avar)
    out = np.concatenate([amean, astd], axis=1).astype(np.float32)
    return out[:, :, None]
